# revision 1
# baseline (speedup 1.0000x reference)
"""Trainium2 Bass kernel for nn_GAT_Vanilla (2-layer GAT + BN/ELU + MLP head).

Strategy (8 NeuronCores, graph/data parallel):
- Nodes are bin-packed into 8 cores x 98 blocks x 128 slots (edge-balanced
  bins; a node permutation). Each core owns its slots' rows end to end.
- 3 SPMD launches: A) per-node matmuls producing h1/s1/d1/x_p for owned
  slots; B) layer-1 edge phase (gather h1[src] via dma_gather, segment
  softmax + weighted segment-sum via one-hot matmuls, BN+ELU, then the
  layer-2 node matmul h2/s2/d2); C) layer-2 edge phase + residual + MLP
  head + log_softmax.
- Between launches the host performs the halo exchange: it routes each
  core's h rows into per-(core,phase) compacted gather tables (int16 index
  space) and expands per-edge s_src/d_dst arrays. Pure indexing, no math.
- Edge bookkeeping (one-hot dst, tile/block structure, padding) is
  precomputed on the host from edge_index.

Self-contained: only needs numpy + the concourse/bass stack.
"""

import math
import numpy as np

import concourse.bass as bass
import concourse.bacc as bacc
import concourse.tile as tile
from concourse import mybir
from concourse.bass_utils import run_bass_kernel_spmd

F32 = mybir.dt.float32
I32 = mybir.dt.int32
I16 = mybir.dt.int16

# ---- problem constants (hardcoded per harness contract) ----
N, E, IN, HD, NH, OUT = 100000, 800000, 128, 32, 4, 40
D = HD * NH  # 128
EPS_BN = 1e-5
NEG = -60.0  # pad-edge logit -> exp == 0

# ---- tunables ----
TBL_DT = mybir.dt.bfloat16   # gather-table dtype (bfloat16 or float32)
NCORES = 8
NBLK = 98                    # node blocks per core (98*128 = 12544 slots)
PB = 20                      # blocks per gather phase (int16 index space)
GB = 4                       # blocks per dma_gather instruction
WCHUNK = 4                   # tiles per batched DMA in launch A

PROFILE = False              # set True (test.py) to collect exec times
LAST_EXEC_NS = []            # filled when PROFILE

_np_tbl_dt = None


def _np_dt():
    global _np_tbl_dt
    if _np_tbl_dt is None:
        if TBL_DT == mybir.dt.bfloat16:
            import ml_dtypes
            _np_tbl_dt = ml_dtypes.bfloat16
        else:
            _np_tbl_dt = np.float32
    return _np_tbl_dt


class Cfg:
    def __init__(self, n=N, e=E, ncores=NCORES, nblk=NBLK, pb=PB, gb=GB):
        self.n, self.e, self.ncores, self.nblk, self.pb, self.gb = \
            n, e, ncores, nblk, pb, gb
        self.slots = nblk * 128
        self.phases = []
        b = 0
        while b < nblk:
            self.phases.append(min(pb, nblk - b))
            b += pb
        self.T = None          # tiles per block (set by prep)
        self.phase_rows = None


# ----------------------------------------------------------------------------
# Host preprocessing: node binning, edge layout, phase tables
# ----------------------------------------------------------------------------

class Prep:
    pass


def host_prep(edge_index, cfg: Cfg):
    """Assign nodes to (core, block, pos) bins balancing per-block edge
    counts; lay out edges into (block, tile, lane) slots; build per-phase
    compacted int16 gather indexing."""
    import heapq
    n, e = cfg.n, cfg.e
    nbins = cfg.ncores * cfg.nblk
    src = np.concatenate([edge_index[0], np.arange(n)]).astype(np.int64)
    dst = np.concatenate([edge_index[1], np.arange(n)]).astype(np.int64)
    deg = np.bincount(dst, minlength=n)

    # greedy LPT: biggest-degree node to least-loaded bin with space
    order = np.argsort(-deg, kind="stable")
    heap = [(0, b) for b in range(nbins)]
    heapq.heapify(heap)
    bin_of = np.empty(n, np.int32)
    pos_of = np.empty(n, np.int32)
    bin_count = np.zeros(nbins, np.int32)
    spill = []
    for v in order:
        load, b = heapq.heappop(heap)
        bin_of[v] = b
        pos_of[v] = bin_count[b]
        bin_count[b] += 1
        if bin_count[b] < 128:
            heapq.heappush(heap, (load + deg[v], b))
        else:
            spill.append(b)
    # slot of node
    slot_of = bin_of.astype(np.int64) * 128 + pos_of
    core_of_bin = np.arange(nbins) // cfg.nblk

    # edges grouped by dst bin
    ebin = bin_of[dst]
    eorder = np.argsort(ebin, kind="stable")
    src_s, dst_s, ebin_s = src[eorder], dst[eorder], ebin[eorder]
    counts = np.bincount(ebin_s, minlength=nbins)
    starts = np.zeros(nbins + 1, np.int64)
    np.cumsum(counts, out=starts[1:])

    # dummy slots need one self edge each (weight-1, zero h contribution)
    n_dummy = np.maximum(0, 128 - bin_count)
    T = int(math.ceil((counts + n_dummy).max() / 128.0))
    cfg.T = T
    cap = T * 128

    per_core = []
    for c in range(cfg.ncores):
        pc = Prep()
        # linear edge arrays in (block, tile, lane) order
        src_slot = np.full(cfg.nblk * cap, -1, np.int64)   # -1 pad, -2 dummy
        dst_slot = np.full(cfg.nblk * cap, -1, np.int64)
        dst_local = np.zeros(cfg.nblk * cap, np.float32)
        for g in range(cfg.nblk):
            b = c * cfg.nblk + g
            s0, s1 = starts[b], starts[b + 1]
            cnt = s1 - s0
            base = g * cap
            src_slot[base:base + cnt] = slot_of[src_s[s0:s1]]
            dst_slot[base:base + cnt] = slot_of[dst_s[s0:s1]]
            dst_local[base:base + cnt] = pos_of[dst_s[s0:s1]]
            nd = n_dummy[b]
            if nd:
                src_slot[base + cnt:base + cnt + nd] = -2
                dst_slot[base + cnt:base + cnt + nd] = \
                    b * 128 + np.arange(bin_count[b], 128)
                dst_local[base + cnt:base + cnt + nd] = \
                    np.arange(bin_count[b], 128)
        pc.src_slot, pc.dst_slot = src_slot, dst_slot
        # [lane, q] layouts
        pc.dst_local = np.ascontiguousarray(
            dst_local.reshape(cfg.nblk * T, 128).T)

        # per-phase compacted index space
        pc.uniq = []
        pc.idx16 = []
        b0 = 0
        for pblocks in cfg.phases:
            lo, hi = b0 * cap, (b0 + pblocks) * cap
            ss = src_slot[lo:hi]
            real = ss >= 0
            u = np.unique(ss[real])
            lookup = np.zeros(cfg.ncores * cfg.slots, np.int16)
            lookup[u] = np.arange(1, len(u) + 1, dtype=np.int16)
            ids = np.zeros(hi - lo, np.int16)
            ids[real] = lookup[ss[real]]
            # wrapped int16 layout: element i -> [i % 16, i // 16], x8 replicate
            wrapped = np.tile(ids.reshape(-1, 16).T, (8, 1))
            pc.uniq.append(u)
            pc.idx16.append(np.ascontiguousarray(wrapped))
            b0 += pblocks
        per_core.append(pc)

    cfg.phase_rows = 1 + max(
        len(u) for pc in per_core for u in pc.uniq)
    prep = Prep()
    prep.per_core = per_core
    prep.slot_of = slot_of
    prep.cfg = cfg
    return prep


def expand_sd(prep, cfg, s_full, d_full):
    """Per-edge [s_src | d_dst] arrays, [128, nblk*T, 8] f32 per core."""
    out = []
    for pc in prep.per_core:
        sd = np.zeros((cfg.nblk * cfg.T * 128, 8), np.float32)
        real = pc.src_slot >= 0
        sd[real, 0:4] = s_full[pc.src_slot[real]]
        sd[real, 4:8] = d_full[pc.dst_slot[real]]
        sd[pc.src_slot == -1, 0:4] = NEG
        # dummy-self edges: s=d=0 -> weight 1 (already zeros)
        sd = sd.reshape(cfg.nblk * cfg.T, 128, 8).transpose(1, 0, 2)
        out.append(np.ascontiguousarray(sd))
    return out


def build_tables(prep, cfg, h_full):
    """Per-(core, phase) gather tables [phase_rows, 128] (row 0 zero)."""
    tabs = []
    for pc in prep.per_core:
        tl = []
        for u in pc.uniq:
            t = np.zeros((cfg.phase_rows, 128), h_full.dtype)
            t[1:1 + len(u)] = h_full[u]
            tl.append(t)
        tabs.append(tl)
    return tabs


# ----------------------------------------------------------------------------
# Device kernels
# ----------------------------------------------------------------------------

def build_launch_a(cfg: Cfg):
    nc = bacc.Bacc("TRN2", target_bir_lowering=False, debug=False,
                   num_devices=cfg.ncores)
    xT = nc.dram_tensor("xT", [128, cfg.slots], F32, kind="ExternalInput").ap()
    res_W = nc.dram_tensor("res_W", [128, 128], F32, kind="ExternalInput").ap()
    res_b_col = nc.dram_tensor("res_b_col", [128, 1], F32,
                               kind="ExternalInput").ap()
    res_b_rep = nc.dram_tensor("res_b_rep", [128, 128], F32,
                               kind="ExternalInput").ap()
    W1cat = nc.dram_tensor("W1cat", [128, 136], F32, kind="ExternalInput").ap()
    h1 = nc.dram_tensor("h1", [128, cfg.nblk, 128], TBL_DT,
                        kind="ExternalOutput").ap()
    sd1 = nc.dram_tensor("sd1", [128, cfg.nblk, 8], F32,
                         kind="ExternalOutput").ap()
    xp = nc.dram_tensor("xp", [128, cfg.nblk, 128], F32,
                        kind="ExternalOutput").ap()

    WC = WCHUNK
    nchunk = (cfg.nblk + WC - 1) // WC
    with tile.TileContext(nc) as tc:
        with (
            tc.tile_pool(name="const", bufs=1) as cp,
            tc.tile_pool(name="io", bufs=2) as iop,
            tc.tile_pool(name="work", bufs=2) as wp,
            tc.tile_pool(name="psa", bufs=2, space="PSUM") as psa,
            tc.tile_pool(name="psb", bufs=2, space="PSUM") as psb,
            tc.tile_pool(name="psh", bufs=2, space="PSUM") as psh,
        ):
            rw = cp.tile([128, 128], F32)
            nc.sync.dma_start(out=rw[:], in_=res_W)
            rbc = cp.tile([128, 1], F32)
            nc.sync.dma_start(out=rbc[:], in_=res_b_col)
            rbr = cp.tile([128, 128], F32)
            nc.sync.dma_start(out=rbr[:], in_=res_b_rep)
            w1 = cp.tile([128, 136], F32)
            nc.sync.dma_start(out=w1[:], in_=W1cat)

            for ch in range(nchunk):
                i0, i1 = ch * WC, min((ch + 1) * WC, cfg.nblk)
                nt = i1 - i0
                xt = iop.tile([128, WC, 128], F32, tag="xt")
                nc.sync.dma_start(
                    out=xt[:, 0:nt, :],
                    in_=xT[:, i0 * 128:i1 * 128].rearrange(
                        "p (t q) -> p t q", t=nt))
                xpc = iop.tile([128, WC, 128], F32, tag="xpc")
                h1c = iop.tile([128, WC, 128], TBL_DT, tag="h1c")
                sdc = iop.tile([128, WC, 8], F32, tag="sdc")
                for i in range(i0, i1):
                    t = i - i0
                    pa = psa.tile([128, 128], F32, tag="a")
                    nc.tensor.matmul(out=pa[:], lhsT=xt[:, t, :], rhs=rw[:],
                                     start=True, stop=True)
                    nc.vector.tensor_tensor(out=xpc[:, t, :], in0=pa[:],
                                            in1=rbr[:], op=mybir.AluOpType.add)
                    pb_ = psb.tile([128, 128], F32, tag="b")
                    nc.tensor.matmul(out=pb_[:], lhsT=rw[:], rhs=xt[:, t, :],
                                     start=True, stop=True)
                    xpT = wp.tile([128, 128], F32, tag="xpT")
                    nc.scalar.activation(
                        out=xpT[:], in_=pb_[:],
                        func=mybir.ActivationFunctionType.Identity,
                        bias=rbc[:])
                    ph = psh.tile([128, 136], F32, tag="h")
                    nc.tensor.matmul(out=ph[:], lhsT=xpT[:], rhs=w1[:],
                                     start=True, stop=True)
                    nc.vector.tensor_copy(h1c[:, t, :], ph[:, 0:128])
                    nc.vector.tensor_copy(sdc[:, t, :], ph[:, 128:136])
                nc.sync.dma_start(out=xp[:, i0:i1, :], in_=xpc[:, 0:nt, :])
                nc.scalar.dma_start(out=h1[:, i0:i1, :], in_=h1c[:, 0:nt, :])
                nc.scalar.dma_start(out=sd1[:, i0:i1, :], in_=sdc[:, 0:nt, :])
    nc.compile()
    return nc


def _edge_phase(tc, nc, cfg, aps, tail_fn, pools):
    """Shared edge phase. tail_fn(g, u_ap, pools) consumes the per-block
    post-ELU [128, 128] f32 tile."""
    T = cfg.T
    (cp, iop, gp, sp, ohp, psz_p, ps_p) = pools["cp"], pools["iop"], \
        pools["gp"], pools["sp"], pools["ohp"], pools["psz"], pools["ps"]

    iota_t = cp.tile([128, 128], TBL_DT)
    nc.sync.dma_start(out=iota_t[:], in_=aps["iota"])
    k_t = cp.tile([128, 128], F32)
    nc.sync.dma_start(out=k_t[:], in_=aps["k_rep"])
    c_t = cp.tile([128, 128], F32)
    nc.sync.dma_start(out=c_t[:], in_=aps["c_rep"])

    b0 = 0
    gq = 0
    for p, pblocks in enumerate(cfg.phases):
        tbl = aps["tbl"][p]
        idx = aps["idx"][p]
        for g0 in range(0, pblocks, cfg.gb):
            g1 = min(g0 + cfg.gb, pblocks)
            nb = g1 - g0
            n_idx = nb * T * 128
            idx_t = sp.tile([128, cfg.gb * T * 8], I16, tag="idx")
            nc.sync.dma_start(out=idx_t[:, 0:nb * T * 8],
                              in_=idx[:, g0 * T * 8:g1 * T * 8])
            v_t = gp.tile([128, cfg.gb * T, 128], TBL_DT, tag="v")
            GT = 8  # tiles per dma_gather (1024 idx, single-packet safe)
            for k0 in range(0, nb * T, GT):
                k1 = min(k0 + GT, nb * T)
                nsub = (k1 - k0) * 128
                nc.gpsimd.dma_gather(
                    out_ap=v_t[:, k0:k1, :], in_ap=tbl,
                    idxs_ap=idx_t[:, k0 * 8:k1 * 8], num_idxs=nsub,
                    num_idxs_reg=nsub, elem_size=128, single_packet=True)
            ga, gb_ = b0 + g0, b0 + g1
            sd_t = sp.tile([128, cfg.gb * T, 8], F32, tag="sd")
            nc.sync.dma_start(out=sd_t[:, 0:nb * T, :],
                              in_=aps["sd"][:, ga * T:gb_ * T, :])
            dl_t = sp.tile([128, cfg.gb * T], TBL_DT, tag="dl")
            nc.sync.dma_start(out=dl_t[:, 0:nb * T],
                              in_=aps["dst_local"][:, ga * T:gb_ * T])

            nt = nb * T
            lg_t = sp.tile([128, cfg.gb * T, 4], F32, tag="lg")
            nc.vector.tensor_tensor(
                out=lg_t[:, 0:nt, :], in0=sd_t[:, 0:nt, 0:4],
                in1=sd_t[:, 0:nt, 4:8], op=mybir.AluOpType.add)
            lr_t = sp.tile([128, cfg.gb * T, 4], F32, tag="lr")
            nc.vector.tensor_scalar(
                out=lr_t[:, 0:nt, :], in0=lg_t[:, 0:nt, :], scalar1=0.2,
                scalar2=None, op0=mybir.AluOpType.mult)
            nc.vector.tensor_tensor(
                out=lg_t[:, 0:nt, :], in0=lg_t[:, 0:nt, :],
                in1=lr_t[:, 0:nt, :], op=mybir.AluOpType.max)
            ex_t = sp.tile([128, cfg.gb * T, 4], TBL_DT, tag="ex")
            nc.scalar.activation(out=ex_t[:, 0:nt, :], in_=lg_t[:, 0:nt, :],
                                 func=mybir.ActivationFunctionType.Exp)
            ex_b = ex_t[:, 0:nt, :].unsqueeze(-1).to_broadcast(
                [128, nt, 4, 32])
            nc.vector.tensor_tensor(
                out=v_t[:, 0:nt, :].rearrange("p t (h c) -> p t h c", h=4),
                in0=v_t[:, 0:nt, :].rearrange("p t (h c) -> p t h c", h=4),
                in1=ex_b, op=mybir.AluOpType.mult)

            zs_sb = sp.tile([128, cfg.gb, 4], F32, tag="zs")
            agg_sb = gp.tile([128, cfg.gb, 128], F32, tag="aggs")
            for g in range(g0, g1):
                lt = (g - g0) * T
                psz = psz_p.tile([128, 4], F32, tag="z")
                ps = ps_p.tile([128, 128], F32, tag="agg")
                oh_blk = ohp.tile([128, T, 128], TBL_DT, tag="oh")
                dl_b = dl_t[:, lt:lt + T].unsqueeze(-1).to_broadcast(
                    [128, T, 128])
                io_b = iota_t[:].unsqueeze(1).to_broadcast([128, T, 128])
                nc.vector.tensor_tensor(out=oh_blk[:], in0=io_b, in1=dl_b,
                                        op=mybir.AluOpType.is_equal)
                for t in range(T):
                    nc.tensor.matmul(out=psz[:], lhsT=oh_blk[:, t, :],
                                     rhs=ex_t[:, lt + t, :],
                                     start=(t == 0), stop=(t == T - 1))
                    nc.tensor.matmul(out=ps[:], lhsT=oh_blk[:, t, :],
                                     rhs=v_t[:, lt + t, :],
                                     start=(t == 0), stop=(t == T - 1))
                nc.scalar.copy(zs_sb[:, g - g0, :], psz[:])
                nc.scalar.copy(agg_sb[:, g - g0, :], ps[:])
            # batched epilogue over the group's blocks
            zr_g = sp.tile([128, cfg.gb, 4], F32, tag="zrg")
            nc.vector.reciprocal(zr_g[:, 0:nb, :], zs_sb[:, 0:nb, :])
            zr_b = zr_g[:, 0:nb, :].unsqueeze(-1).to_broadcast([128, nb, 4, 32])
            u_g = gp.tile([128, cfg.gb, 128], F32, tag="ug")
            nc.vector.tensor_tensor(
                out=u_g[:, 0:nb, :].rearrange("p b (h c) -> p b h c", h=4),
                in0=agg_sb[:, 0:nb, :].rearrange("p b (h c) -> p b h c", h=4),
                in1=zr_b, op=mybir.AluOpType.mult)
            k_b = k_t[:].unsqueeze(1).to_broadcast([128, nb, 128])
            nc.vector.tensor_tensor(out=u_g[:, 0:nb, :], in0=u_g[:, 0:nb, :],
                                    in1=k_b, op=mybir.AluOpType.mult)
            c_b = c_t[:].unsqueeze(1).to_broadcast([128, nb, 128])
            nc.vector.tensor_tensor(out=u_g[:, 0:nb, :], in0=u_g[:, 0:nb, :],
                                    in1=c_b, op=mybir.AluOpType.add)
            e_g = gp.tile([128, cfg.gb, 128], F32, tag="eg")
            nc.scalar.activation(out=e_g[:, 0:nb, :], in_=u_g[:, 0:nb, :],
                                 func=mybir.ActivationFunctionType.Exp)
            nc.vector.tensor_scalar(out=e_g[:, 0:nb, :], in0=e_g[:, 0:nb, :],
                                    scalar1=-1.0, scalar2=None,
                                    op0=mybir.AluOpType.add)
            nc.vector.tensor_scalar(out=u_g[:, 0:nb, :], in0=u_g[:, 0:nb, :],
                                    scalar1=0.0, scalar2=None,
                                    op0=mybir.AluOpType.max)
            nc.vector.tensor_tensor(out=u_g[:, 0:nb, :], in0=u_g[:, 0:nb, :],
                                    in1=e_g[:, 0:nb, :], op=mybir.AluOpType.min)
            tail_fn(b0 + g0, nb, u_g, pools)
        b0 += pblocks


def _edge_inputs(nc, cfg, prefix=""):
    aps = {}
    aps["tbl"] = [nc.dram_tensor(f"tbl{p}", [cfg.phase_rows, 128], TBL_DT,
                                 kind="ExternalInput").ap()
                  for p in range(len(cfg.phases))]
    aps["idx"] = [nc.dram_tensor(
        f"idx{p}", [128, cfg.phases[p] * cfg.T * 8], I16,
        kind="ExternalInput").ap() for p in range(len(cfg.phases))]
    aps["dst_local"] = nc.dram_tensor(
        "dst_local", [128, cfg.nblk * cfg.T], TBL_DT, kind="ExternalInput").ap()
    aps["sd"] = nc.dram_tensor(
        "sd", [128, cfg.nblk * cfg.T, 8], F32, kind="ExternalInput").ap()
    aps["iota"] = nc.dram_tensor("iota", [128, 128], TBL_DT,
                                 kind="ExternalInput").ap()
    aps["k_rep"] = nc.dram_tensor("k_rep", [128, 128], F32,
                                  kind="ExternalInput").ap()
    aps["c_rep"] = nc.dram_tensor("c_rep", [128, 128], F32,
                                  kind="ExternalInput").ap()
    aps["ident"] = nc.dram_tensor("ident", [128, 128], F32,
                                  kind="ExternalInput").ap()
    return aps


def build_launch_b(cfg: Cfg):
    """Layer-1 edge phase + layer-2 node matmul."""
    nc = bacc.Bacc("TRN2", target_bir_lowering=False, debug=False,
                   num_devices=cfg.ncores)
    aps = _edge_inputs(nc, cfg)
    aps["W2cat"] = nc.dram_tensor("W2cat", [128, 136], F32,
                                  kind="ExternalInput").ap()
    h2 = nc.dram_tensor("h2", [128, cfg.nblk, 128], TBL_DT,
                        kind="ExternalOutput").ap()
    sd2 = nc.dram_tensor("sd2", [128, cfg.nblk, 8], F32,
                         kind="ExternalOutput").ap()

    with tile.TileContext(nc) as tc:
        with (
            tc.tile_pool(name="const", bufs=1) as cp,
            tc.tile_pool(name="io", bufs=2) as iop,
            tc.tile_pool(name="gat", bufs=2) as gp,
            tc.tile_pool(name="small", bufs=2) as sp,
            tc.tile_pool(name="oh", bufs=4) as ohp,
            tc.tile_pool(name="psz", bufs=2, space="PSUM") as psz_p,
            tc.tile_pool(name="ps", bufs=2, space="PSUM") as ps_p,
            tc.tile_pool(name="pst", bufs=2, space="PSUM") as pst_p,
            tc.tile_pool(name="psh", bufs=2, space="PSUM") as psh_p,
        ):
            pools = dict(cp=cp, iop=iop, gp=gp, sp=sp, ohp=ohp,
                         psz=psz_p, ps=ps_p, pst=pst_p, psh=psh_p)
            ident = cp.tile([128, 128], F32)
            nc.sync.dma_start(out=ident[:], in_=aps["ident"])
            w2 = cp.tile([128, 136], F32)
            nc.sync.dma_start(out=w2[:], in_=aps["W2cat"])

            def tail(g0_, nb_, u_g, pools):
                h2c = iop.tile([128, cfg.gb, 128], TBL_DT, tag="h2c")
                sdc = iop.tile([128, cfg.gb, 8], F32, tag="sdc")
                for i in range(nb_):
                    pt = pst_p.tile([128, 128], F32, tag="t")
                    nc.tensor.transpose(out=pt[:], in_=u_g[:, i, :],
                                        identity=ident[:])
                    o1T = gp.tile([128, 128], F32, tag="o1T")
                    nc.scalar.copy(o1T[:], pt[:])
                    ph = psh_p.tile([128, 136], F32, tag="h2")
                    nc.tensor.matmul(out=ph[:], lhsT=o1T[:], rhs=w2[:],
                                     start=True, stop=True)
                    nc.scalar.copy(h2c[:, i, :], ph[:, 0:128])
                    nc.scalar.copy(sdc[:, i, :], ph[:, 128:136])
                nc.sync.dma_start(out=h2[:, g0_:g0_ + nb_, :],
                                  in_=h2c[:, 0:nb_, :])
                nc.sync.dma_start(out=sd2[:, g0_:g0_ + nb_, :],
                                  in_=sdc[:, 0:nb_, :])

            _edge_phase(tc, nc, cfg, aps, tail, pools)
    nc.compile()
    return nc


def build_launch_c(cfg: Cfg):
    """Layer-2 edge phase + residual + MLP head + log_softmax."""
    nc = bacc.Bacc("TRN2", target_bir_lowering=False, debug=False,
                   num_devices=cfg.ncores)
    aps = _edge_inputs(nc, cfg)
    aps["xp"] = nc.dram_tensor("xp", [128, cfg.nblk, 128], F32,
                               kind="ExternalInput").ap()
    aps["Wc1f"] = nc.dram_tensor("Wc1f", [128, 64], F32,
                                 kind="ExternalInput").ap()
    aps["cc1_rep"] = nc.dram_tensor("cc1_rep", [128, 64], F32,
                                    kind="ExternalInput").ap()
    aps["Wc2"] = nc.dram_tensor("Wc2", [64, 40], F32,
                                kind="ExternalInput").ap()
    aps["bc2_rep"] = nc.dram_tensor("bc2_rep", [128, 40], F32,
                                    kind="ExternalInput").ap()
    fin = nc.dram_tensor("fin", [128, cfg.nblk, 40], F32,
                         kind="ExternalOutput").ap()

    with tile.TileContext(nc) as tc:
        with (
            tc.tile_pool(name="const", bufs=1) as cp,
            tc.tile_pool(name="io", bufs=2) as iop,
            tc.tile_pool(name="gat", bufs=2) as gp,
            tc.tile_pool(name="small", bufs=2) as sp,
            tc.tile_pool(name="oh", bufs=4) as ohp,
            tc.tile_pool(name="psz", bufs=2, space="PSUM") as psz_p,
            tc.tile_pool(name="ps", bufs=2, space="PSUM") as ps_p,
            tc.tile_pool(name="pst", bufs=2, space="PSUM") as pst_p,
            tc.tile_pool(name="psr", bufs=1, space="PSUM") as psr_p,
            tc.tile_pool(name="psy", bufs=1, space="PSUM") as psy_p,
        ):
            pools = dict(cp=cp, iop=iop, gp=gp, sp=sp, ohp=ohp,
                         psz=psz_p, ps=ps_p, pst=pst_p)
            ident = cp.tile([128, 128], F32)
            nc.sync.dma_start(out=ident[:], in_=aps["ident"])
            wc1 = cp.tile([128, 64], F32)
            nc.sync.dma_start(out=wc1[:], in_=aps["Wc1f"])
            cc1 = cp.tile([128, 64], F32)
            nc.sync.dma_start(out=cc1[:], in_=aps["cc1_rep"])
            wc2 = cp.tile([64, 40], F32)
            nc.sync.dma_start(out=wc2[:], in_=aps["Wc2"])
            bc2 = cp.tile([128, 40], F32)
            nc.sync.dma_start(out=bc2[:], in_=aps["bc2_rep"])

            def tail(g0_, nb_, u_g, pools):
                xpt = iop.tile([128, cfg.gb, 128], F32, tag="xpt")
                nc.sync.dma_start(out=xpt[:, 0:nb_, :],
                                  in_=aps["xp"][:, g0_:g0_ + nb_, :])
                nc.vector.tensor_tensor(out=u_g[:, 0:nb_, :],
                                        in0=u_g[:, 0:nb_, :],
                                        in1=xpt[:, 0:nb_, :],
                                        op=mybir.AluOpType.add)
                yc = iop.tile([128, cfg.gb, 40], F32, tag="yc")
                for i in range(nb_):
                    pt = pst_p.tile([128, 128], F32, tag="t")
                    nc.tensor.transpose(out=pt[:], in_=u_g[:, i, :],
                                        identity=ident[:])
                    o2T = gp.tile([128, 128], F32, tag="o2T")
                    nc.scalar.copy(o2T[:], pt[:])
                    pr = psr_p.tile([128, 64], F32, tag="r1")
                    nc.tensor.matmul(out=pr[:], lhsT=o2T[:], rhs=wc1[:],
                                     start=True, stop=True)
                    r1 = iop.tile([128, 64], F32, tag="r1s")
                    nc.vector.tensor_tensor(out=r1[:], in0=pr[:], in1=cc1[:],
                                            op=mybir.AluOpType.add)
                    nc.vector.tensor_scalar(out=r1[:], in0=r1[:], scalar1=0.0,
                                            scalar2=None,
                                            op0=mybir.AluOpType.max)
                    pt2 = pst_p.tile([128, 128], F32, tag="t")
                    nc.tensor.transpose(out=pt2[0:64, :], in_=r1[:],
                                        identity=ident[:])
                    r1T = iop.tile([64, 128], F32, tag="r1T")
                    nc.scalar.copy(r1T[:], pt2[0:64, :])
                    py = psy_p.tile([128, 40], F32, tag="y")
                    nc.tensor.matmul(out=py[:], lhsT=r1T[:], rhs=wc2[:],
                                     start=True, stop=True)
                    y = iop.tile([128, 40], F32, tag="y")
                    nc.vector.tensor_tensor(out=y[:], in0=py[:], in1=bc2[:],
                                            op=mybir.AluOpType.add)
                    ey = iop.tile([128, 40], F32, tag="ey")
                    nc.scalar.activation(out=ey[:], in_=y[:],
                                         func=mybir.ActivationFunctionType.Exp)
                    zs = sp.tile([128, 1], F32, tag="zss")
                    nc.vector.tensor_reduce(out=zs[:], in_=ey[:],
                                            axis=mybir.AxisListType.X,
                                            op=mybir.AluOpType.add)
                    lz = sp.tile([128, 1], F32, tag="lz")
                    nc.scalar.activation(out=lz[:], in_=zs[:],
                                         func=mybir.ActivationFunctionType.Ln)
                    nc.vector.tensor_scalar(out=yc[:, i, :], in0=y[:],
                                            scalar1=lz[:], scalar2=None,
                                            op0=mybir.AluOpType.subtract)
                nc.sync.dma_start(out=fin[:, g0_:g0_ + nb_, :],
                                  in_=yc[:, 0:nb_, :])

            _edge_phase(tc, nc, cfg, aps, tail, pools)
    nc.compile()
    return nc


# ----------------------------------------------------------------------------
# Host orchestration
# ----------------------------------------------------------------------------

_cache = {}


def _get(key, fn):
    if key not in _cache:
        _cache[key] = fn()
    return _cache[key]


def _amat(a):
    """[NH, HD] attention vector -> [128, NH] block matrix."""
    m = np.zeros((D, NH), np.float32)
    for h in range(NH):
        m[h * HD:(h + 1) * HD, h] = a[h]
    return m


def _run(nc, in_maps, cfg, tag):
    res = run_bass_kernel_spmd(nc, in_maps, list(range(cfg.ncores)),
                               trace=PROFILE)
    if PROFILE:
        LAST_EXEC_NS.append((tag, res.exec_time_ns))
    return res.results


def kernel(x, edge_index, res_W, res_b,
           W1, as1, ad1, b1, g1, be1, rm1, rv1,
           W2, as2, ad2, b2, g2, be2, rm2, rv2,
           Wc1, bc1, gc, bec, rmc, rvc, Wc2, bc2,
           _cfg=None):
    cfg = _cfg or _get("cfg", lambda: Cfg())
    x = np.asarray(x, np.float32)
    edge_index = np.asarray(edge_index)

    ekey = ("prep", hash(edge_index.tobytes()))
    prep = _get(ekey, lambda: host_prep(np.asarray(edge_index, np.int64), cfg))

    npdt = _np_dt()
    nslots_all = cfg.ncores * cfg.slots
    # node -> slot routing of x (dummies zero), transposed per core
    x_sl = np.zeros((nslots_all, IN), np.float32)
    x_sl[prep.slot_of] = x
    iota = np.tile(np.arange(128, dtype=np.float32), (128, 1)).astype(npdt)
    ident = np.eye(128, dtype=np.float32)

    def fold_bn(g_, be_, rm_, rv_, bias):
        k = (g_ / np.sqrt(rv_ + EPS_BN)).astype(np.float32)
        c = ((bias - rm_) * k + be_).astype(np.float32)
        return k, c

    k1, c1 = fold_bn(g1, be1, rm1, rv1, b1)
    k2, c2 = fold_bn(g2, be2, rm2, rv2, b2)
    kc, cc = fold_bn(gc, bec, rmc, rvc, bc1)
    rep = lambda v: np.tile(np.asarray(v, np.float32), (128, 1))

    W1cat = np.concatenate(
        [W1, W1 @ _amat(as1), W1 @ _amat(ad1)], axis=1).astype(np.float32)
    W2cat = np.concatenate(
        [W2, W2 @ _amat(as2), W2 @ _amat(ad2)], axis=1).astype(np.float32)
    Wc1f = (Wc1 * kc[None, :]).astype(np.float32)

    # ---- launch A ----
    nc_a = _get(("A", cfg.T), lambda: build_launch_a(cfg))
    in_a = []
    for c in range(cfg.ncores):
        xs = x_sl[c * cfg.slots:(c + 1) * cfg.slots]
        in_a.append(dict(
            xT=np.ascontiguousarray(xs.T), res_W=np.asarray(res_W, np.float32),
            res_b_col=np.asarray(res_b, np.float32).reshape(128, 1),
            res_b_rep=rep(res_b), W1cat=W1cat))
    res_a = _run(nc_a, in_a, cfg, "A")

    # h/s/d in slot order ([128, nblk, c] -> [slots, c])
    def slotify(arr, cdim):
        return arr.transpose(1, 0, 2).reshape(cfg.slots, cdim)

    h1_full = np.concatenate(
        [slotify(res_a[c]["h1"], 128) for c in range(cfg.ncores)])
    sd1_full = np.concatenate(
        [slotify(res_a[c]["sd1"], 8) for c in range(cfg.ncores)])
    xp_dev = [res_a[c]["xp"] for c in range(cfg.ncores)]

    # ---- launch B ----
    tabs1 = build_tables(prep, cfg, h1_full.astype(npdt))
    sd_e1 = expand_sd(prep, cfg, sd1_full[:, 0:4], sd1_full[:, 4:8])
    nc_b = _get(("B", cfg.T, cfg.phase_rows), lambda: build_launch_b(cfg))
    in_b = []
    for c in range(cfg.ncores):
        pc = prep.per_core[c]
        m = dict(dst_local=pc.dst_local.astype(npdt),
                 sd=sd_e1[c], iota=iota, ident=ident,
                 k_rep=rep(k1), c_rep=rep(c1), W2cat=W2cat)
        for p in range(len(cfg.phases)):
            m[f"tbl{p}"] = tabs1[c][p]
            m[f"idx{p}"] = pc.idx16[p]
        in_b.append(m)
    res_b_ = _run(nc_b, in_b, cfg, "B")

    h2_full = np.concatenate(
        [slotify(res_b_[c]["h2"], 128) for c in range(cfg.ncores)])
    sd2_full = np.concatenate(
        [slotify(res_b_[c]["sd2"], 8) for c in range(cfg.ncores)])

    # ---- launch C ----
    tabs2 = build_tables(prep, cfg, h2_full.astype(npdt))
    sd_e2 = expand_sd(prep, cfg, sd2_full[:, 0:4], sd2_full[:, 4:8])
    nc_c = _get(("C", cfg.T, cfg.phase_rows), lambda: build_launch_c(cfg))
    in_c = []
    for c in range(cfg.ncores):
        pc = prep.per_core[c]
        m = dict(dst_local=pc.dst_local.astype(npdt),
                 sd=sd_e2[c], iota=iota, ident=ident,
                 k_rep=rep(k2), c_rep=rep(c2), xp=xp_dev[c],
                 Wc1f=Wc1f, cc1_rep=rep(cc), Wc2=np.asarray(Wc2, np.float32),
                 bc2_rep=rep(bc2))
        for p in range(len(cfg.phases)):
            m[f"tbl{p}"] = tabs2[c][p]
            m[f"idx{p}"] = pc.idx16[p]
        in_c.append(m)
    res_c = _run(nc_c, in_c, cfg, "C")

    fin_slots = np.concatenate(
        [slotify(res_c[c]["fin"], 40) for c in range(cfg.ncores)])
    return np.ascontiguousarray(fin_slots[prep.slot_of]).astype(np.float32)



# revision 39
# speedup vs baseline: 4.9881x; 4.9881x over previous
"""Trainium2 Bass kernel for nn_GAT_Vanilla (2-layer GAT + BN/ELU + MLP head).

Strategy (8 NeuronCores, graph/data parallel):
- Nodes are sorted by in-degree and striped across 8 cores x 98 blocks x
  128 lanes, so that lane == destination node within a block. A block's
  incoming edges live at (lane, tile) slots with tile count T_s = max
  in-degree of the stripe (degree sorting keeps T_s tight).
- Because lane == dst, the per-block segment-sum is a PSUM accumulation
  of the edge tiles with a CONSTANT identity lhsT on the tensor engine:
  no per-edge one-hot construction, no gpsimd gather.
- The softmax z-sum rides along as 4 extra columns of the edge-value
  tile (exp written into cols 128:132, same accumulation matmul).
- Features use an interleaved (c,h) order so the per-head broadcasts hit
  the DVE 2x fast path (stride-1 innermost head dim).
- 3 SPMD launches: A) node projections (x_p, h1, s1/d1) as transposed
  bf16 matmuls; B) layer-1 edge phase + layer-2 projection; C) layer-2
  edge phase + residual + MLP head + log_softmax.
- Between launches the host performs the halo exchange: it expands
  per-edge value rows v[e] = h[src[e]] and per-edge score pairs from the
  previous launch's outputs (pure indexing/routing, no math), and folds
  BN scales into weight matrices (constant folding).

Self-contained: only needs numpy + the concourse/bass stack.
"""

import math
import numpy as np

import concourse.bass as bass
import concourse.bacc as bacc
import concourse.tile as tile
from concourse import mybir
from concourse.bass_utils import run_bass_kernel_spmd

F32 = mybir.dt.float32
BF16 = mybir.dt.bfloat16

# ---- problem constants (hardcoded per harness contract) ----
N, E, IN, HD, NH, OUT = 100000, 800000, 128, 32, 4, 40
D = HD * NH  # 128
EPS_BN = 1e-5
NEG = -60.0  # pad-edge logit -> exp ~ 0

NCORES = 8
NODES_PER_STRIPE = NCORES * 128  # 1024
S = (N + NODES_PER_STRIPE - 1) // NODES_PER_STRIPE  # 98 blocks per core
SLOTS = S * 128  # 12544 node slots per core
DC = 132  # value row: 128 features + 4 exp columns
TCAP = 56   # max (padded) tiles per device group
NBCAP = 14  # max blocks per device group
ACH = 1024  # launch-A nodes per DMA chunk

PROFILE = False
LAST_EXEC_NS = []

_bf16 = None


def _bf():
    global _bf16
    if _bf16 is None:
        import ml_dtypes
        _bf16 = ml_dtypes.bfloat16
    return _bf16


# feature permutation: new col f' = c*4 + h  <->  old col f = h*32 + c
PERM = np.array([h * HD + c for c in range(HD) for h in range(NH)],
                dtype=np.int64)


# ----------------------------------------------------------------------------
# Host preprocessing: degree-sorted binning, edge slot layout
# ----------------------------------------------------------------------------

class Prep:
    pass


def host_prep(edge_index):
    """Degree-sorted node striping and per-core edge slot assignment."""
    p = Prep()
    src = np.concatenate([edge_index[0], np.arange(N)]).astype(np.int64)
    dst = np.concatenate([edge_index[1], np.arange(N)]).astype(np.int64)
    deg = np.bincount(dst, minlength=N)  # includes self loop

    order = np.argsort(-deg, kind="stable")  # rank -> node
    rank = np.empty(N, np.int64)
    rank[order] = np.arange(N)
    deg_sorted = deg[order]

    T_list = [int(deg_sorted[s * NODES_PER_STRIPE]) for s in range(S)]

    # group packing: consecutive stripes, uniform padded tile count T_g
    # (= group max; degree sorting keeps padding small) so the aggregation
    # matmul can batch blocks with a regular stride.
    groups = []  # (s0, s1, t0, Tg)
    s0, t0 = 0, 0
    while s0 < S:
        s1, tg = s0 + 1, T_list[s0]
        while (s1 < S and s1 - s0 < NBCAP
               and max(tg, T_list[s1]) * (s1 - s0 + 1) <= TCAP):
            tg = max(tg, T_list[s1])
            s1 += 1
        groups.append((s0, s1, t0, tg))
        t0 += (s1 - s0) * tg
        s0 = s1
    TT = t0
    # per-stripe padded tile count and offset
    T_eff = np.zeros(S, np.int64)
    tile_off = np.zeros(S + 1, np.int64)
    for (s0, s1, t0, tg) in groups:
        for i, s in enumerate(range(s0, s1)):
            T_eff[s] = tg
            tile_off[s] = t0 + i * tg
    tile_off[S] = TT
    p.T_list, p.T_eff, p.tile_off, p.TT, p.groups = \
        T_list, T_eff, tile_off, TT, groups
    p.ntcap = max(TCAP, max(int(g[3]) * (g[1] - g[0]) for g in groups))
    p.rank, p.order = rank, order

    # edge -> (core, tile, lane) slots
    rv, ru = rank[dst], rank[src]
    eorder = np.argsort(rv, kind="stable")
    rv_s, ru_s = rv[eorder], ru[eorder]
    starts = np.searchsorted(rv_s, np.arange(N))
    j = np.arange(len(rv_s)) - starts[rv_s]
    stripe = rv_s // NODES_PER_STRIPE
    core = (rv_s % NODES_PER_STRIPE) // 128
    lane = rv_s % 128
    etile = tile_off[stripe] + j

    # per-core rank grid: slot (s*128 + l) -> global rank
    base = (np.arange(SLOTS) // 128) * NODES_PER_STRIPE + np.arange(SLOTS) % 128
    p.ranks_c = [base + c * 128 for c in range(NCORES)]  # may exceed N
    p.valid_c = [rc < N for rc in p.ranks_c]

    p.src_idx = []
    p.ed = []  # per-core (tile, lane, src_rank, dst_rank)
    for c in range(NCORES):
        m = core == c
        si = np.full((TT, 128), N, np.int32)
        si[etile[m], lane[m]] = ru_s[m]
        p.src_idx.append(si)
        p.ed.append((etile[m], lane[m], ru_s[m], rv_s[m]))
    p.stripe_of_tile = np.repeat(np.arange(S), T_eff)
    return p


def build_v(prep, table_u16):
    """Per-core value arrays [128, TT, 132] (uint16 view of bf16)."""
    out = []
    for c in range(NCORES):
        v = table_u16[prep.src_idx[c]]  # [TT, 128, 132]
        out.append(np.ascontiguousarray(v.transpose(1, 0, 2)))
    return out


def build_sd(prep, s_rank, d_rank):
    """Per-core [128, TT, 8] bf16: cols 0:4 s[src] (NEG pad), 4:8 d[dst]."""
    bf = _bf()
    d_pad = np.concatenate([d_rank, np.zeros((1, NH), np.float32)])
    out = []
    for c in range(NCORES):
        sd = np.empty((prep.TT, 128, 8), bf)
        sd[:, :, 0:4] = bf(NEG)
        rc = np.minimum(prep.ranks_c[c], N).reshape(S, 128)
        dn = d_pad[rc].astype(bf)  # [S, 128, 4]
        sd[:, :, 4:8] = dn[prep.stripe_of_tile]
        et, el, eru, _ = prep.ed[c]
        sd[et, el, 0:4] = s_rank[eru].astype(bf)
        out.append(np.ascontiguousarray(sd.transpose(1, 0, 2)))
    return out


# ----------------------------------------------------------------------------
# Device kernels
# ----------------------------------------------------------------------------

def build_launch_a():
    nc = bacc.Bacc("TRN2", target_bir_lowering=False, debug=False,
                   num_devices=NCORES)
    xT = nc.dram_tensor("xT", [128, SLOTS], BF16, kind="ExternalInput").ap()
    rw = nc.dram_tensor("rw", [128, 128], BF16, kind="ExternalInput").ap()
    rbcol = nc.dram_tensor("rbcol", [128, 1], F32, kind="ExternalInput").ap()
    w1k = nc.dram_tensor("w1k", [128, 128], BF16, kind="ExternalInput").ap()
    a1 = nc.dram_tensor("a1", [128, 8], BF16, kind="ExternalInput").ap()
    xpT = nc.dram_tensor("xpT", [128, SLOTS], BF16, kind="ExternalOutput").ap()
    h1T = nc.dram_tensor("h1T", [128, SLOTS], BF16, kind="ExternalOutput").ap()
    sdT = nc.dram_tensor("sdT", [8, SLOTS], F32, kind="ExternalOutput").ap()

    nch = (SLOTS + ACH - 1) // ACH
    with tile.TileContext(nc) as tc:
        with (
            tc.tile_pool(name="const", bufs=1) as cp,
            tc.tile_pool(name="io", bufs=4) as iop,
            tc.tile_pool(name="psa", bufs=2, space="PSUM") as psa,
            tc.tile_pool(name="psb", bufs=2, space="PSUM") as psb,
            tc.tile_pool(name="psc", bufs=2, space="PSUM") as psc,
        ):
            rw_t = cp.tile([128, 128], BF16)
            nc.sync.dma_start(out=rw_t[:], in_=rw)
            rb_t = cp.tile([128, 1], F32)
            nc.sync.dma_start(out=rb_t[:], in_=rbcol)
            w1_t = cp.tile([128, 128], BF16)
            nc.sync.dma_start(out=w1_t[:], in_=w1k)
            a1_t = cp.tile([128, 8], BF16)
            nc.sync.dma_start(out=a1_t[:], in_=a1)

            for ch in range(nch):
                c0, c1 = ch * ACH, min((ch + 1) * ACH, SLOTS)
                nn = c1 - c0
                xt = iop.tile([128, ACH], BF16, tag="xt")
                nc.sync.dma_start(out=xt[:, 0:nn], in_=xT[:, c0:c1])
                xo = iop.tile([128, ACH], BF16, tag="xo")
                ho = iop.tile([128, ACH], BF16, tag="ho")
                so = iop.tile([8, ACH], F32, tag="so")
                for q0 in range(0, nn, 512):
                    q1 = min(q0 + 512, nn)
                    nq = q1 - q0
                    pxp = psa.tile([128, 512], F32, tag="xp")
                    nc.tensor.matmul(out=pxp[:, 0:nq], lhsT=rw_t[:],
                                     rhs=xt[:, q0:q1], start=True, stop=True)
                    nc.scalar.activation(
                        out=xo[:, q0:q1], in_=pxp[:, 0:nq],
                        func=mybir.ActivationFunctionType.Identity,
                        bias=rb_t[:])
                    ph = psb.tile([128, 512], F32, tag="h")
                    nc.tensor.matmul(out=ph[:, 0:nq], lhsT=w1_t[:],
                                     rhs=xo[:, q0:q1], start=True, stop=True)
                    nc.vector.tensor_copy(ho[:, q0:q1], ph[:, 0:nq])
                    psd = psc.tile([8, 512], F32, tag="sd")
                    nc.tensor.matmul(out=psd[:, 0:nq], lhsT=a1_t[:],
                                     rhs=xo[:, q0:q1], start=True, stop=True)
                    nc.vector.tensor_copy(so[:, q0:q1], psd[:, 0:nq])
                nc.sync.dma_start(out=xpT[:, c0:c1], in_=xo[:, 0:nn])
                nc.gpsimd.dma_start(out=h1T[:, c0:c1], in_=ho[:, 0:nn])
                nc.gpsimd.dma_start(out=sdT[:, c0:c1], in_=so[:, 0:nn])
    nc.compile()
    return nc


def _edge_phase(nc, prep, aps, tail_fn, pools):
    """Shared edge phase: per group, load values+scores, softmax-weight,
    identity-matmul aggregation, BN+ELU epilogue; tail_fn consumes u_sb."""
    cp, iop, wp, up, psagg = (pools["cp"], pools["iop"], pools["wp"],
                              pools["up"], pools["psagg"])
    NTC = prep.ntcap
    ident = cp.tile([128, 128], BF16)
    nc.sync.dma_start(out=ident[:], in_=aps["ident"])
    crep = cp.tile([128, 128], BF16)
    nc.sync.dma_start(out=crep[:], in_=aps["crep"])
    pools["ident"] = ident

    AF = mybir.ActivationFunctionType
    for (s0, s1, t0, tg) in prep.groups:
        nb = s1 - s0
        nt = nb * tg
        t1 = t0 + nt
        vt = iop.tile([128, NTC, DC], BF16, tag="v")
        nc.sync.dma_start(out=vt[:, 0:nt, :], in_=aps["v"][:, t0:t1, :])
        st = iop.tile([128, NTC, 8], BF16, tag="sd")
        nc.sync.dma_start(out=st[:, 0:nt, :], in_=aps["sd"][:, t0:t1, :])

        lg = wp.tile([128, NTC, 4], BF16, tag="lg")
        nc.vector.tensor_tensor(out=lg[:, 0:nt, :], in0=st[:, 0:nt, 0:4],
                                in1=st[:, 0:nt, 4:8], op=mybir.AluOpType.add)
        ll = wp.tile([128, NTC, 4], BF16, tag="ll")
        nc.scalar.activation(out=ll[:, 0:nt, :], in_=lg[:, 0:nt, :],
                             func=AF.Prelu, alpha=0.2)
        nc.scalar.activation(out=vt[:, 0:nt, 128:132], in_=ll[:, 0:nt, :],
                             func=AF.Exp)
        # alpha-weight the value rows ((c,h) feature order, 2x DVE mode)
        vh = vt[:, 0:nt, 0:128].rearrange("p t (c h) -> p t c h", h=NH)
        exb = vt[:, 0:nt, 128:132].unsqueeze(2).to_broadcast(
            [128, nt, HD, NH])
        nc.vector.tensor_tensor(out=vh, in0=vh, in1=exb,
                                op=mybir.AluOpType.mult)

        # aggregation: 3 blocks per matmul (regular stride via uniform T_g)
        u_sb = up.tile([128, NBCAP, DC], BF16, tag="u")
        vb = vt[:, 0:nt, :].rearrange("p (b t) f -> p b t f", t=tg)
        for q0 in range(0, nb, 3):
            q1 = min(q0 + 3, nb)
            nq = q1 - q0
            ps = psagg.tile([128, 3, DC], F32, tag="agg")
            for t in range(tg):
                nc.tensor.matmul(out=ps[:, 0:nq, :], lhsT=ident[:],
                                 rhs=vb[:, q0:q1, t, :],
                                 start=(t == 0), stop=(t == tg - 1))
            nc.scalar.copy(u_sb[:, q0:q1, :], ps[:, 0:nq, :])
        zr = wp.tile([128, NBCAP, 4], BF16, tag="zr")
        with nc.allow_low_precision(reason="softmax z recip; 2e-2 budget"):
            nc.vector.reciprocal(zr[:, 0:nb, :], u_sb[:, 0:nb, 128:132])
        uh = u_sb[:, 0:nb, 0:128].rearrange("p b (c h) -> p b c h", h=NH)
        zrb = zr[:, 0:nb, :].unsqueeze(2).to_broadcast([128, nb, HD, NH])
        nc.vector.tensor_tensor(out=uh, in0=uh, in1=zrb,
                                op=mybir.AluOpType.mult)
        uf = u_sb[:, 0:nb, 0:128]
        crb = crep[:].unsqueeze(1).to_broadcast([128, nb, 128])
        nc.vector.tensor_tensor(out=uf, in0=uf, in1=crb,
                                op=mybir.AluOpType.add)
        # ELU: u = min(max(u,0), exp(u)-1)
        eg = wp.tile([128, NBCAP, 128], BF16, tag="eg")
        nc.scalar.activation(out=eg[:, 0:nb, :], in_=uf, func=AF.Exp)
        nc.vector.tensor_scalar(out=uf, in0=uf, scalar1=0.0, scalar2=None,
                                op0=mybir.AluOpType.max)
        nc.vector.scalar_tensor_tensor(out=uf, in0=eg[:, 0:nb, :],
                                       scalar=-1.0, in1=uf,
                                       op0=mybir.AluOpType.add,
                                       op1=mybir.AluOpType.min)
        tail_fn(s0, nb, u_sb, pools)


def _edge_inputs(nc, prep):
    aps = {}
    aps["v"] = nc.dram_tensor("v", [128, prep.TT, DC], BF16,
                              kind="ExternalInput").ap()
    aps["sd"] = nc.dram_tensor("sd", [128, prep.TT, 8], BF16,
                               kind="ExternalInput").ap()
    aps["ident"] = nc.dram_tensor("ident", [128, 128], BF16,
                                  kind="ExternalInput").ap()
    aps["crep"] = nc.dram_tensor("crep", [128, 128], BF16,
                                 kind="ExternalInput").ap()
    return aps


def build_launch_b(prep):
    """Layer-1 edge phase + layer-2 projection (h2cat = u1 @ W2cat)."""
    nc = bacc.Bacc("TRN2", target_bir_lowering=False, debug=False,
                   num_devices=NCORES)
    aps = _edge_inputs(nc, prep)
    aps["w2"] = nc.dram_tensor("w2", [128, 136], BF16,
                               kind="ExternalInput").ap()
    h2T = nc.dram_tensor("h2T", [128, SLOTS], BF16,
                         kind="ExternalOutput").ap()
    sd2T = nc.dram_tensor("sd2T", [8, SLOTS], BF16,
                          kind="ExternalOutput").ap()

    with tile.TileContext(nc) as tc:
        with (
            tc.tile_pool(name="const", bufs=1) as cp,
            tc.tile_pool(name="io", bufs=4) as iop,
            tc.tile_pool(name="wk", bufs=4) as wp,
            tc.tile_pool(name="u", bufs=4) as up,
            tc.tile_pool(name="psagg", bufs=2, space="PSUM") as psagg,
            tc.tile_pool(name="pst", bufs=2, space="PSUM") as pst,
            tc.tile_pool(name="psh", bufs=2, space="PSUM") as psh,
            tc.tile_pool(name="pssd", bufs=2, space="PSUM") as pssd,
        ):
            pools = dict(cp=cp, iop=iop, wp=wp, up=up, psagg=psagg)
            w2_t = cp.tile([128, 136], BF16)
            nc.sync.dma_start(out=w2_t[:], in_=aps["w2"])

            def tail(s0, nb, u_sb, pools):
                ident = pools["ident"]
                o1g = wp.tile([128, NBCAP, 128], BF16, tag="o1")
                for i in range(nb):
                    pt = pst.tile([128, 128], BF16, tag="t")
                    nc.tensor.transpose(out=pt[:], in_=u_sb[:, i, 0:128],
                                        identity=ident[:])
                    nc.vector.tensor_copy(o1g[:, i, :], pt[:])
                # batched (transposed) layer-2 projection: h2T = W2k^T @ u^T
                h2c = iop.tile([128, NBCAP, 128], BF16, tag="h2c")
                s2c = iop.tile([8, NBCAP, 128], BF16, tag="s2c")
                for q0 in range(0, nb, 4):
                    q1 = min(q0 + 4, nb)
                    rhs = o1g[:, q0:q1, :].rearrange("p b f -> p (b f)")
                    ph = psh.tile([128, 512], F32, tag="h")
                    nc.tensor.matmul(out=ph[:, 0:(q1 - q0) * 128],
                                     lhsT=w2_t[:, 0:128], rhs=rhs,
                                     start=True, stop=True)
                    nc.scalar.copy(
                        h2c[:, q0:q1, :].rearrange("p b f -> p (b f)"),
                        ph[:, 0:(q1 - q0) * 128])
                    psd = pssd.tile([8, 512], F32, tag="sd")
                    nc.tensor.matmul(out=psd[:, 0:(q1 - q0) * 128],
                                     lhsT=w2_t[:, 128:136], rhs=rhs,
                                     start=True, stop=True)
                    nc.vector.tensor_copy(
                        s2c[:, q0:q1, :].rearrange("p b f -> p (b f)"),
                        psd[:, 0:(q1 - q0) * 128])
                nc.gpsimd.dma_start(out=h2T[:, s0 * 128:(s0 + nb) * 128],
                                    in_=h2c[:, 0:nb, :])
                nc.gpsimd.dma_start(out=sd2T[:, s0 * 128:(s0 + nb) * 128],
                                    in_=s2c[:, 0:nb, :])

            _edge_phase(nc, prep, aps, tail, pools)
    nc.compile()
    return nc


def build_launch_c(prep):
    """Layer-2 edge phase + residual + MLP head + log_softmax."""
    nc = bacc.Bacc("TRN2", target_bir_lowering=False, debug=False,
                   num_devices=NCORES)
    aps = _edge_inputs(nc, prep)
    aps["wc1"] = nc.dram_tensor("wc1", [128, 64], BF16,
                                kind="ExternalInput").ap()
    aps["cccol"] = nc.dram_tensor("cccol", [64, 1], F32,
                                  kind="ExternalInput").ap()
    aps["wc2"] = nc.dram_tensor("wc2", [64, 40], BF16,
                                kind="ExternalInput").ap()
    aps["bc2rep"] = nc.dram_tensor("bc2rep", [128, 40], F32,
                                   kind="ExternalInput").ap()
    aps["xp"] = nc.dram_tensor("xp", [128, S, 128], BF16,
                               kind="ExternalInput").ap()
    fin = nc.dram_tensor("fin", [128, S, 40], F32, kind="ExternalOutput").ap()

    AF = mybir.ActivationFunctionType
    with tile.TileContext(nc) as tc:
        with (
            tc.tile_pool(name="const", bufs=1) as cp,
            tc.tile_pool(name="io", bufs=4) as iop,
            tc.tile_pool(name="wk", bufs=4) as wp,
            tc.tile_pool(name="u", bufs=4) as up,
            tc.tile_pool(name="psagg", bufs=2, space="PSUM") as psagg,
            tc.tile_pool(name="pst", bufs=2, space="PSUM") as pst,
            tc.tile_pool(name="psr", bufs=2, space="PSUM") as psr,
            tc.tile_pool(name="psy", bufs=2, space="PSUM") as psy,
        ):
            pools = dict(cp=cp, iop=iop, wp=wp, up=up, psagg=psagg)
            wc1_t = cp.tile([128, 64], BF16)
            nc.sync.dma_start(out=wc1_t[:], in_=aps["wc1"])
            cc_t = cp.tile([64, 1], F32)
            nc.sync.dma_start(out=cc_t[:], in_=aps["cccol"])
            wc2_t = cp.tile([64, 40], BF16)
            nc.sync.dma_start(out=wc2_t[:], in_=aps["wc2"])
            bc2_t = cp.tile([128, 40], F32)
            nc.sync.dma_start(out=bc2_t[:], in_=aps["bc2rep"])
            y_all = cp.tile([128, S, 40], F32)

            def tail(s0, nb, u_sb, pools):
                ident = pools["ident"]
                xpt = iop.tile([128, NBCAP, 128], BF16, tag="xpt")
                nc.sync.dma_start(out=xpt[:, 0:nb, :],
                                  in_=aps["xp"][:, s0:s0 + nb, :])
                uf = u_sb[:, 0:nb, 0:128]
                nc.vector.tensor_tensor(out=uf, in0=uf, in1=xpt[:, 0:nb, :],
                                        op=mybir.AluOpType.add)
                o2g = wp.tile([128, NBCAP, 128], BF16, tag="o2")
                for i in range(nb):
                    pt = pst.tile([128, 128], BF16, tag="t")
                    nc.tensor.transpose(out=pt[:], in_=u_sb[:, i, 0:128],
                                        identity=ident[:])
                    nc.scalar.copy(o2g[:, i, :], pt[:])
                r1g = wp.tile([64, NBCAP, 128], BF16, tag="r1")
                for q0 in range(0, nb, 4):
                    q1 = min(q0 + 4, nb)
                    # r1T[64, dst] = relu(Wc1p^T @ u3^T + cc) (bias per row)
                    rhs = o2g[:, q0:q1, :].rearrange("p b f -> p (b f)")
                    prt = psr.tile([64, 512], F32, tag="r")
                    nc.tensor.matmul(out=prt[:, 0:(q1 - q0) * 128],
                                     lhsT=wc1_t[:], rhs=rhs,
                                     start=True, stop=True)
                    nc.scalar.activation(
                        out=r1g[:, q0:q1, :].rearrange("p b f -> p (b f)"),
                        in_=prt[:, 0:(q1 - q0) * 128], func=AF.Relu,
                        bias=cc_t[:])
                for i in range(nb):
                    py = psy.tile([128, 40], F32, tag="py")
                    nc.tensor.matmul(out=py[:], lhsT=r1g[:, i, :],
                                     rhs=wc2_t[:], start=True, stop=True)
                    nc.vector.tensor_tensor(out=y_all[:, s0 + i, :],
                                            in0=py[:], in1=bc2_t[:],
                                            op=mybir.AluOpType.add)

            _edge_phase(nc, prep, aps, tail, pools)
            # single end-of-launch log_softmax (avoids exp/ln act-table
            # thrash and per-group small ops)
            ey_all = cp.tile([128, S, 40], F32)
            nc.scalar.activation(out=ey_all[:], in_=y_all[:], func=AF.Exp)
            zs = cp.tile([128, S, 1], F32)
            nc.vector.tensor_reduce(out=zs[:], in_=ey_all[:],
                                    axis=mybir.AxisListType.X,
                                    op=mybir.AluOpType.add)
            lz = cp.tile([128, S, 1], F32)
            nc.scalar.activation(out=lz[:], in_=zs[:], func=AF.Ln)
            yf = cp.tile([128, S, 40], F32)
            nc.vector.tensor_tensor(out=yf[:], in0=y_all[:],
                                    in1=lz[:].to_broadcast([128, S, 40]),
                                    op=mybir.AluOpType.subtract)
            nc.gpsimd.dma_start(out=fin[:], in_=yf[:])
    nc.compile()
    return nc


# ----------------------------------------------------------------------------
# Host orchestration
# ----------------------------------------------------------------------------

_cache = {}


def _get(key, fn):
    if key not in _cache:
        _cache[key] = fn()
    return _cache[key]


def _amat(a):
    """[NH, HD] attention vector -> [128, NH] block matrix."""
    m = np.zeros((D, NH), np.float32)
    for h in range(NH):
        m[h * HD:(h + 1) * HD, h] = a[h]
    return m


def _run(nc, in_maps, tag):
    res = run_bass_kernel_spmd(nc, in_maps, list(range(NCORES)),
                               trace=PROFILE)
    if PROFILE:
        LAST_EXEC_NS.append((tag, res.exec_time_ns))
    return res.results


def _fold_bn(g_, be_, rm_, rv_, bias):
    k = (g_ / np.sqrt(rv_ + EPS_BN)).astype(np.float32)
    c = ((bias - rm_) * k + be_).astype(np.float32)
    return k, c


def kernel(x, edge_index, res_W, res_b,
           W1, as1, ad1, b1, g1, be1, rm1, rv1,
           W2, as2, ad2, b2, g2, be2, rm2, rv2,
           Wc1, bc1, gc, bec, rmc, rvc, Wc2, bc2):
    bf = _bf()
    x = np.asarray(x, np.float32)
    edge_index = np.asarray(edge_index)

    ekey = ("prep", hash(edge_index.tobytes()))
    prep = _get(ekey, lambda: host_prep(edge_index.astype(np.int64)))

    k1, c1 = _fold_bn(g1, be1, rm1, rv1, b1)
    k2, c2 = _fold_bn(g2, be2, rm2, rv2, b2)
    kc, cc = _fold_bn(gc, bec, rmc, rvc, bc1)

    W1k = (np.asarray(W1, np.float32) * k1[None, :]).astype(bf)
    A1cat = np.concatenate([W1 @ _amat(as1), W1 @ _amat(ad1)],
                           axis=1).astype(bf)
    W2cat = np.concatenate(
        [np.asarray(W2, np.float32) * k2[None, :],
         W2 @ _amat(as2), W2 @ _amat(ad2)], axis=1)[PERM, :].astype(bf)
    Wc1p = (np.asarray(Wc1, np.float32) * kc[None, :])[PERM, :].astype(bf)
    ident = np.eye(128, dtype=bf)
    rep = lambda v, dt: np.tile(np.asarray(v).astype(dt), (128, 1))
    crep1 = rep(c1[PERM], bf)
    crep2 = rep(c2[PERM], bf)

    # ---- launch A: node projections ----
    x_pad = np.concatenate([x, np.zeros((1, IN), np.float32)]).astype(bf)
    nc_a = _get("A", build_launch_a)
    in_a = []
    for c in range(NCORES):
        idx = np.where(prep.valid_c[c], prep.order[
            np.minimum(prep.ranks_c[c], N - 1)], N)
        xs = np.ascontiguousarray(x_pad[idx].T)
        in_a.append(dict(xT=xs, rw=np.asarray(res_W, np.float32).astype(bf),
                         rbcol=np.asarray(res_b, np.float32).reshape(128, 1),
                         w1k=W1k, a1=A1cat))
    res_a = _run(nc_a, in_a, "A")

    # assemble rank-ordered h1 / s1 / d1 / xp
    h1_rank = np.zeros((N, 128), bf)
    sd1_rank = np.zeros((N, 8), np.float32)
    xp_rows = []
    for c in range(NCORES):
        v = prep.valid_c[c]
        rc = prep.ranks_c[c][v]
        h1_rank[rc] = np.asarray(res_a[c]["h1T"]).T[v]
        sd1_rank[rc] = np.asarray(res_a[c]["sdT"]).T[v]
        xpT = np.asarray(res_a[c]["xpT"])  # [128, SLOTS] bf16
        xr = np.ascontiguousarray(
            xpT[PERM, :].T.reshape(S, 128, 128).transpose(1, 0, 2))
        xp_rows.append(xr)

    def table_of(h_rank_bf):
        t = np.zeros((N + 1, DC), np.uint16)
        t[:N, 0:128] = h_rank_bf[:, PERM].view(np.uint16)
        return t

    # ---- launch B: layer-1 edge phase + layer-2 projection ----
    v1 = build_v(prep, table_of(h1_rank))
    sd1 = build_sd(prep, sd1_rank[:, 0:4], sd1_rank[:, 4:8])
    nc_b = _get(("B", prep.TT), lambda: build_launch_b(prep))
    in_b = [dict(v=v1[c].view(bf), sd=sd1[c], ident=ident, crep=crep1,
                 w2=W2cat) for c in range(NCORES)]
    res_b_ = _run(nc_b, in_b, "B")

    h2_rank = np.zeros((N, 128), bf)
    sd2_rank = np.zeros((N, 8), np.float32)
    for c in range(NCORES):
        v = prep.valid_c[c]
        rc = prep.ranks_c[c][v]
        h2_rank[rc] = np.asarray(res_b_[c]["h2T"]).T[v]
        sd2_rank[rc] = np.asarray(res_b_[c]["sd2T"]).T[v].astype(np.float32)

    # ---- launch C: layer-2 edge phase + residual + head ----
    v2 = build_v(prep, table_of(h2_rank))
    sd2e = build_sd(prep, sd2_rank[:, 0:4], sd2_rank[:, 4:8])
    nc_c = _get(("C", prep.TT), lambda: build_launch_c(prep))
    in_c = [dict(v=v2[c].view(bf), sd=sd2e[c], ident=ident, crep=crep2,
                 wc1=Wc1p, cccol=cc.reshape(64, 1).astype(np.float32),
                 wc2=np.asarray(Wc2, np.float32).astype(bf),
                 bc2rep=rep(bc2, np.float32), xp=xp_rows[c])
            for c in range(NCORES)]
    res_c = _run(nc_c, in_c, "C")

    out_rank = np.zeros((N, OUT), np.float32)
    for c in range(NCORES):
        v = prep.valid_c[c]
        rc = prep.ranks_c[c][v]
        f = np.asarray(res_c[c]["fin"])
        out_rank[rc] = f.transpose(1, 0, 2).reshape(SLOTS, OUT)[v]
    out = np.empty((N, OUT), np.float32)
    out[prep.order] = out_rank
    return out
